# revision 42
# baseline (speedup 1.0000x reference)
"""AdaLN Trainium2 kernel v15 — raw Bass, replicated weights, 137us.

Data-parallel over batch: each of the 8 cores runs one batch element
(x [4096,1024] in/out streamed, w/b/g replicated; no collectives).
Key design points (each validated against a perfetto trace):

  - DMA: the 8 weight chunks are issued FIRST (SP descriptor generation
    costs ~1us per dma_start, and queue striping is FIFO per queue), so
    swinv is ready ~35us; the x ring (NX=18) streams behind them.
    Per-slot semaphores everywhere: DMA completions are NOT ordered
    across dma_starts, so cumulative counts on a single semaphore race.
  - c arrives twice: as a [1,1024] row (absmax) and as [8,128] rows
    that a PE transpose turns into the [128,8] matmul operand (the
    naive [128,8] DMA gather is 1024 descriptors and poisons the head).
  - w-quant via bf16-write rounding: u = bf16(w*swinv + 192) rounds RNE
    to integers (ULP=1 in [128,256)); one 4x-mode DVE clamp to [191,193]
    finishes it.  The 192 offset is NOT subtracted: the matmul runs on
    wq+192 and the epilogue folds -192*sum(cq)*os into b_row (error
    ~0.03%: the f32 PSUM partial sums only random-walk to ~1e6).  Magic
    passes alternate ACT (odd chunks) / DVE (even chunks).
  - emb matmul: cqi bf16 x ternary bf16, accumulated over 8 chunks in
    4 PSUM banks; epilogue folds the (amax/127)*mean|w| scale and bias,
    a = (emb_scale+1)*g and shift rows are broadcast to 128 partitions
    via PE ones-matmuls (bf16 rhs = single PE pass).
  - modulation stream per [128,1024] tile: ACT does square+accum, Ln,
    Exp (one act-table covers all funcs; tiles 0-2 pre-run in the idle
    window before swinv) and z = x*inv for 18 of 32 tiles (Copy with
    per-partition AP scale); DVE does the fused stt (x*inv)*A -> bf16
    for the rest, y = z*A for ACT-z tiles, and out = y + shift (f32)
    for every tile, software-pipelined one tile behind the stt/y.  GPSIMD stays IDLE: it
    shares SBUF ports with DVE and concurrent GPS work slows DVE ~3x
    (this is why the v5 stt measured 3.2us; it is 1.2us GPS-free).
  - outputs stage through wt_sb (dead after w-quant) so the out-DMA
    and the next x-load never touch the same buffer.
"""

import sys
from contextlib import ExitStack

import numpy as np

sys.path.insert(0, "/opt/trn_rl_repo")
sys.path.insert(0, "/opt/pypackages")

import concourse.bass as bass
from concourse import mybir
from concourse.bass_utils import run_bass_kernel_spmd

F32 = mybir.dt.float32
BF16 = mybir.dt.bfloat16
ALU = mybir.AluOpType
ACTF = mybir.ActivationFunctionType

P = 128
D = 1024
CD = 1024
DD = 2 * D
B = 8
S_FULL = 4096

EPS_RMS = 1e-6
EPS_Q = 1e-5
MAGIC = 1.5 * 2.0**23  # f32 round-to-int trick (c quant)
MBF = 192.0            # bf16 round-to-int magic (w quant): ULP=1 in [128,256)
CHI = 193.0
CLO = 191.0


KC_G = CD // P


def build(S=S_FULL, NX=20, NZ=4, NU=4):
    nc = bass.Bass()

    x_d = nc.declare_dram_parameter("x", [S, D], F32, isOutput=False)
    c_d = nc.declare_dram_parameter("c", [CD], F32, isOutput=False)
    wt_d = nc.declare_dram_parameter("wt", [CD, DD], F32, isOutput=False)
    b_d = nc.declare_dram_parameter("b", [DD], F32, isOutput=False)
    g_d = nc.declare_dram_parameter("g", [D], F32, isOutput=False)
    id8_d = nc.declare_dram_parameter("id8", [KC_G, KC_G], F32, isOutput=False)
    out_d = nc.declare_dram_parameter("out", [S, D], F32, isOutput=True)

    KC = CD // P
    NT = S // P
    NX = min(NX, NT)
    NS = min(20, NT)

    NY = KC  # output staging ring carved from wt_sb (dead after w-quant)

    def act_owns_z(i):
        # 18 of 32 tiles on ACT, evenly spread (balances ACT vs DVE ends)
        return (i * 18) % 32 < 18

    def n_zA(j):  # count of ACT-z tiles <= j
        return sum(1 for t in range(j + 1) if act_owns_z(t))

    ctx = ExitStack()
    with ctx:
        # ---------------- SBUF ----------------
        ones = ctx.enter_context(nc.sbuf_tensor("ones", [P, P], F32))
        obf = ctx.enter_context(nc.sbuf_tensor("obf", [1, P], BF16))
        id8 = ctx.enter_context(nc.sbuf_tensor("id8_sb", [KC, KC], F32))
        eps_t = ctx.enter_context(nc.sbuf_tensor("eps", [P, 1], F32))
        wt_sb = ctx.enter_context(nc.sbuf_tensor("wt_sb", [P, KC, DD], F32))
        red = ctx.enter_context(nc.sbuf_tensor("red", [P, KC], F32))
        sw = ctx.enter_context(nc.sbuf_tensor("sw", [P, 1], F32))
        swa = ctx.enter_context(nc.sbuf_tensor("swa", [P, 1], F32))
        mp = ctx.enter_context(nc.sbuf_tensor("mp", [P, 1], F32))
        swinv = ctx.enter_context(nc.sbuf_tensor("swinv", [P, 1], F32))
        u = [
            ctx.enter_context(nc.sbuf_tensor(f"u{j}", [P, DD], BF16))
            for j in range(NU)
        ]
        c_row = ctx.enter_context(nc.sbuf_tensor("c_row", [1, CD], F32))
        c8 = ctx.enter_context(nc.sbuf_tensor("c8", [KC, P], F32))
        ct = ctx.enter_context(nc.sbuf_tensor("ct", [P, KC], F32))
        am = ctx.enter_context(nc.sbuf_tensor("am", [1, 1], F32))
        amc = ctx.enter_context(nc.sbuf_tensor("amc", [1, 1], F32))
        rc = ctx.enter_context(nc.sbuf_tensor("rc", [1, 1], F32))
        r127 = ctx.enter_context(nc.sbuf_tensor("r127", [1, 1], F32))
        r127_b = ctx.enter_context(nc.sbuf_tensor("r127_b", [P, 1], F32))
        cqt = ctx.enter_context(nc.sbuf_tensor("cqt", [P, KC], F32))
        cqi = ctx.enter_context(nc.sbuf_tensor("cqi", [P, KC], BF16))
        osx = ctx.enter_context(nc.sbuf_tensor("osx", [1, 1], F32))
        cqsum = ctx.enter_context(nc.sbuf_tensor("cqsum", [P, 1], F32))
        sb_s = ctx.enter_context(nc.sbuf_tensor("sb_s", [1, 1], F32))
        offt = ctx.enter_context(nc.sbuf_tensor("offt", [1, 1], F32))
        offv = ctx.enter_context(nc.sbuf_tensor("offv", [1, 1], F32))
        os_t = ctx.enter_context(nc.sbuf_tensor("os_t", [1, 1], F32))
        b_row = ctx.enter_context(nc.sbuf_tensor("b_row", [1, DD], F32))
        g_row = ctx.enter_context(nc.sbuf_tensor("g_row", [1, D], F32))
        emb = ctx.enter_context(nc.sbuf_tensor("emb", [1, D], F32))
        a_bf = ctx.enter_context(nc.sbuf_tensor("a_bf", [1, D], BF16))
        s_bf = ctx.enter_context(nc.sbuf_tensor("s_bf", [1, D], BF16))
        a_bc = ctx.enter_context(nc.sbuf_tensor("a_bc", [P, D], BF16))
        a_fc = ctx.enter_context(nc.sbuf_tensor("a_fc", [P, D], F32))
        b_bc = ctx.enter_context(nc.sbuf_tensor("b_bc", [P, D], BF16))

        xt = [
            ctx.enter_context(nc.sbuf_tensor(f"xt{j}", [P, D], F32))
            for j in range(NX)
        ]
        zt = [
            ctx.enter_context(nc.sbuf_tensor(f"zt{j}", [P, D], BF16))
            for j in range(NZ)
        ]
        sq = ctx.enter_context(nc.sbuf_tensor("sqs", [P, D], F32))
        ss = [
            ctx.enter_context(nc.sbuf_tensor(f"ss{j}", [P, 1], F32))
            for j in range(NS)
        ]
        std = [
            ctx.enter_context(nc.sbuf_tensor(f"std{j}", [P, 1], F32))
            for j in range(NS)
        ]
        inv = [
            ctx.enter_context(nc.sbuf_tensor(f"inv{j}", [P, 1], F32))
            for j in range(NS)
        ]

        par_ps = ctx.enter_context(nc.psum_tensor("par_ps", [P, 16], F32))
        ct_ps = par_ps[:, 8:16]
        emb_ps = ctx.enter_context(nc.psum_tensor("emb_ps", [1, 4, 512], F32))
        bc_ps = [
            ctx.enter_context(nc.psum_tensor(f"bc_ps{j}", [P, 512], F32))
            for j in range(3)
        ]

        # ---------------- semaphores ----------------
        sem_pre = ctx.enter_context(nc.semaphore("pre"))
        sem_c1 = ctx.enter_context(nc.semaphore("c1"))
        sem_c8 = ctx.enter_context(nc.semaphore("c8s"))
        sem_vec = ctx.enter_context(nc.semaphore("vec"))
        sem_wk = [ctx.enter_context(nc.semaphore(f"wk{k}")) for k in range(KC)]
        sem_xt = [ctx.enter_context(nc.semaphore(f"xt{j}")) for j in range(NX)]
        sem_ot = [ctx.enter_context(nc.semaphore(f"ot{j}")) for j in range(NY)]
        sem_tp = ctx.enter_context(nc.semaphore("tps"))
        sem_r127 = ctx.enter_context(nc.semaphore("r127s"))
        sem_pe1 = ctx.enter_context(nc.semaphore("pe1"))
        sem_pe2 = ctx.enter_context(nc.semaphore("pe2"))
        sem_swcp = ctx.enter_context(nc.semaphore("swcp"))
        sem_cq = ctx.enter_context(nc.semaphore("cqs"))
        sem_redA = ctx.enter_context(nc.semaphore("redA"))
        sem_sw = ctx.enter_context(nc.semaphore("sws"))
        sem_qrdy = ctx.enter_context(nc.semaphore("qrdy"))
        sem_cqs = ctx.enter_context(nc.semaphore("cqss"))
        sem_pe3 = ctx.enter_context(nc.semaphore("pe3"))
        sem_off = ctx.enter_context(nc.semaphore("offs"))
        sem_brow = ctx.enter_context(nc.semaphore("brow"))
        sem_mg = ctx.enter_context(nc.semaphore("mg"))
        sem_wq = ctx.enter_context(nc.semaphore("wq"))
        sem_mmk = ctx.enter_context(nc.semaphore("mmk"))
        sem_mmh = ctx.enter_context(nc.semaphore("mmh"))
        sem_emb = ctx.enter_context(nc.semaphore("embs"))
        sem_embB = ctx.enter_context(nc.semaphore("embB"))
        sem_bcmm = ctx.enter_context(nc.semaphore("bcmm"))
        sem_bccp = ctx.enter_context(nc.semaphore("bccp"))
        sem_std = ctx.enter_context(nc.semaphore("stds"))
        sem_zA = ctx.enter_context(nc.semaphore("zA"))
        sem_add = ctx.enter_context(nc.semaphore("adds"))

        wt_r = wt_d[:].rearrange("(k p) n -> k p n", p=P)

        with nc.Block() as block:

            # ================= SP =================
            @block.sync
            def _(sync):
                # weight chunks first (SP descriptor-gen is ~1us per
                # dma_start, so anything before them delays the whole left
                # edge), then the small rows, then the x ring
                for k in range(KC):
                    sync.dma_start(out=wt_sb[:, k, :], in_=wt_r[k, :, :]).then_inc(
                        sem_wk[k], 16
                    )
                sync.dma_start(out=c_row[:], in_=c_d[None, :]).then_inc(sem_c1, 16)
                sync.dma_start(
                    out=c8[:], in_=c_d[:].rearrange("(k p) -> k p", p=P)
                ).then_inc(sem_c8, 16)
                sync.dma_start(out=id8[:], in_=id8_d[:]).then_inc(sem_c8, 16)
                sync.dma_start(out=b_row[:], in_=b_d[None, :]).then_inc(sem_vec, 16)
                sync.dma_start(out=g_row[:], in_=g_d[None, :]).then_inc(sem_vec, 16)
                for j in range(NX):
                    sync.dma_start(
                        out=xt[j][:], in_=x_d[j * P : (j + 1) * P, :]
                    ).then_inc(sem_xt[j], 16)
                for i in range(NT):
                    sync.wait_ge(sem_add, i + 1)
                    sync.dma_start(
                        out=out_d[i * P : (i + 1) * P, :],
                        in_=wt_sb[:, i % NY, 0:D],
                    ).then_inc(sem_ot[i % NY], 16)
                    if i + NX < NT:
                        j = i + NX
                        sync.dma_start(
                            out=xt[j % NX][:], in_=x_d[j * P : (j + 1) * P, :]
                        ).then_inc(sem_xt[j % NX], 16)
                for j in range(NY):
                    cnt = (NT - j + NY - 1) // NY
                    sync.wait_ge(sem_ot[j], 16 * cnt)

            # ================= ACT =================
            @block.scalar
            def _(scalar):
                scalar.wait_ge(sem_pre, 2)
                # per-chunk |w| reduces as the weight DMAs land
                for k in range(KC):
                    scalar.wait_ge(sem_wk[k], 16)
                    scalar.drain()
                    scalar.activation(
                        u[k % NU][:], wt_sb[:, k, :], ACTF.Abs,
                        accum_out=red[:, k : k + 1],
                    ).then_inc(sem_redA, 1)
                # stats pipeline head start: squares/ln/exp for tiles 0-2
                # fill the idle window between the |w| reduces and swinv
                for i in range(3):
                    scalar.wait_ge(sem_xt[i % NX], 16)
                    scalar.drain()
                    scalar.activation(
                        sq[:], xt[i % NX][:], ACTF.Square,
                        accum_out=ss[i % NS][:],
                    )
                    if i >= 1:
                        scalar.activation(
                            std[(i - 1) % NS][:], ss[(i - 1) % NS][:], ACTF.Ln,
                            bias=eps_t[:], scale=1.0 / D,
                        )
                    if i >= 2:
                        scalar.activation(
                            inv[(i - 2) % NS][:], std[(i - 2) % NS][:], ACTF.Exp,
                            scale=-0.5,
                        ).then_inc(sem_std, 1)

                # magic-round passes for chunks 1,3,5,7: u = bf16(w*swinv+192)
                scalar.wait_ge(sem_qrdy, 1)
                for k in range(KC):
                    if k % 2 == 0:
                        continue
                    if k >= NU:
                        scalar.wait_ge(sem_mmk, k - NU + 1)
                    scalar.drain()
                    scalar.activation(
                        u[k % NU][:], wt_sb[:, k, :], ACTF.Copy,
                        bias=MBF, scale=swinv[:],
                    ).then_inc(sem_mg, 1)

                # x statistics stream (+ z for ACT-owned tiles)
                for i in range(3, NT + 3):
                    if i < NT:
                        scalar.wait_ge(sem_xt[i % NX], 16 * (i // NX + 1))
                        if i >= NS:
                            scalar.wait_ge(sem_add, i - NS + 1)
                    scalar.drain()
                    if i < NT:
                        scalar.activation(
                            sq[:], xt[i % NX][:], ACTF.Square,
                            accum_out=ss[i % NS][:],
                        )
                    if 1 <= i <= NT:
                        scalar.activation(
                            std[(i - 1) % NS][:], ss[(i - 1) % NS][:], ACTF.Ln,
                            bias=eps_t[:], scale=1.0 / D,
                        )
                    if 2 <= i <= NT + 1:
                        scalar.activation(
                            inv[(i - 2) % NS][:], std[(i - 2) % NS][:], ACTF.Exp,
                            scale=-0.5,
                        ).then_inc(sem_std, 1)
                    if i >= 3:
                        # z lags exp by one op: inv write has drained
                        j = i - 3
                        if act_owns_z(j):
                            if j >= NZ:
                                scalar.wait_ge(sem_add, j - NZ + 1)
                            scalar.activation(
                                zt[j % NZ][:], xt[j % NX][:], ACTF.Copy,
                                scale=inv[j % NS][:],
                            ).then_inc(sem_zA, 1)

            # ================= DVE =================
            @block.vector
            def _(vector):
                vector.memset(ones[:], 1.0).then_inc(sem_pre, 1)
                vector.memset(eps_t[:], EPS_RMS).then_inc(sem_pre, 1)
                vector.memset(obf[:], 1.0)

                # --- c quant part 1 (absmax scale) ---
                vector.wait_ge(sem_c1, 16)
                vector.tensor_reduce(
                    out=am[:], in_=c_row[:], axis=mybir.AxisListType.X,
                    op=ALU.max, apply_absolute_value=True,
                )
                vector.drain()
                vector.tensor_scalar(
                    out=amc[:], in0=am[:], scalar1=EPS_Q, scalar2=None, op0=ALU.max
                )
                vector.drain()
                vector.reciprocal(rc[:], amc[:])
                vector.drain()
                vector.tensor_scalar(
                    out=r127[:], in0=rc[:], scalar1=127.0, scalar2=None,
                    op0=ALU.mult,
                ).then_inc(sem_r127, 1)
                # --- weight stats tail (before the c-quant epilogue so the
                # PE sw-broadcast -> swinv path never waits on c traffic) ---
                vector.wait_ge(sem_redA, KC)
                vector.tensor_reduce(
                    out=sw[:], in_=red[:], axis=mybir.AxisListType.X, op=ALU.add
                ).then_inc(sem_sw, 1)
                vector.wait_ge(sem_pe2, 1)
                vector.tensor_scalar(
                    out=mp[:], in0=par_ps[:, 0:1], scalar1=1.0 / (CD * DD),
                    scalar2=EPS_Q, op0=ALU.mult, op1=ALU.max,
                ).then_inc(sem_swcp, 1)
                vector.drain()
                vector.reciprocal(swinv[:], mp[:]).then_inc(sem_qrdy, 1)
                vector.drain()
                vector.tensor_tensor(osx[:], amc[:], mp[0:1, :], op=ALU.mult)
                vector.drain()
                vector.tensor_scalar(
                    out=os_t[:], in0=osx[:], scalar1=1.0 / 127.0, scalar2=None,
                    op0=ALU.mult,
                )

                # --- c quant part 2 ---
                vector.wait_ge(sem_pe1, 1)
                vector.tensor_copy(r127_b[:], par_ps[:, 1:2])
                vector.wait_ge(sem_tp, 1)
                vector.tensor_copy(ct[:], ct_ps)
                vector.drain()
                vector.tensor_scalar(
                    out=cqt[:], in0=ct[:], scalar1=r127_b[:], scalar2=MAGIC,
                    op0=ALU.mult, op1=ALU.add,
                )
                vector.drain()
                vector.tensor_scalar(
                    out=cqi[:], in0=cqt[:], scalar1=MAGIC, scalar2=None,
                    op0=ALU.subtract,
                ).then_inc(sem_cq, 1)
                vector.drain()
                vector.tensor_reduce(
                    out=cqsum[:], in_=cqi[:], axis=mybir.AxisListType.X,
                    op=ALU.add,
                ).then_inc(sem_cqs, 1)
                vector.wait_ge(sem_pe3, 1)
                vector.tensor_copy(sb_s[:], par_ps[0:1, 2:3])
                vector.drain()
                vector.tensor_tensor(offt[:], sb_s[:], os_t[:], op=ALU.mult)
                vector.drain()
                vector.tensor_scalar(
                    out=offv[:], in0=offt[:], scalar1=-MBF, scalar2=None,
                    op0=ALU.mult,
                )
                vector.drain()
                vector.wait_ge(sem_vec, 32)
                vector.tensor_scalar(
                    out=b_row[:], in0=b_row[:], scalar1=offv[:], scalar2=None,
                    op0=ALU.add,
                )

                # --- w quant: DVE magic chunk 0, clamp/sub all chunks ---
                for k in range(KC):
                    if k % 2 == 0:
                        if k >= NU:
                            vector.wait_ge(sem_mmk, k - NU + 1)
                        vector.tensor_scalar(
                            out=u[k % NU][:], in0=wt_sb[:, k, :],
                            scalar1=swinv[:], scalar2=MBF,
                            op0=ALU.mult, op1=ALU.add,
                        )
                        vector.drain()
                    else:
                        vector.wait_ge(sem_mg, (k + 1) // 2)
                    vector.tensor_scalar(
                        out=u[k % NU][:], in0=u[k % NU][:], scalar1=CHI,
                        scalar2=CLO, op0=ALU.min, op1=ALU.max,
                    ).then_inc(sem_wq, 1)

                # --- emb epilogue (scale half first, then shift half) ---
                vector.wait_ge(sem_mmh, 1)
                vector.drain()
                vector.scalar_tensor_tensor(
                    out=emb[:, 0:512], in0=emb_ps[:, 0, :], scalar=os_t[:],
                    in1=b_row[:, 0:512], op0=ALU.mult, op1=ALU.add,
                )
                vector.scalar_tensor_tensor(
                    out=emb[:, 512:1024], in0=emb_ps[:, 1, :], scalar=os_t[:],
                    in1=b_row[:, 512:1024], op0=ALU.mult, op1=ALU.add,
                )
                vector.drain()
                vector.scalar_tensor_tensor(
                    out=a_bf[:, 0:512], in0=emb[:, 0:512], scalar=1.0,
                    in1=g_row[:, 0:512], op0=ALU.add, op1=ALU.mult,
                ).then_inc(sem_emb, 1)
                vector.scalar_tensor_tensor(
                    out=a_bf[:, 512:1024], in0=emb[:, 512:1024], scalar=1.0,
                    in1=g_row[:, 512:1024], op0=ALU.add, op1=ALU.mult,
                ).then_inc(sem_emb, 1)
                vector.wait_ge(sem_mmk, KC)
                vector.scalar_tensor_tensor(
                    out=s_bf[:, 0:512], in0=emb_ps[:, 2, :], scalar=os_t[:],
                    in1=b_row[:, 1024:1536], op0=ALU.mult, op1=ALU.add,
                ).then_inc(sem_embB, 1)
                vector.scalar_tensor_tensor(
                    out=s_bf[:, 512:1024], in0=emb_ps[:, 3, :], scalar=os_t[:],
                    in1=b_row[:, 1536:2048], op0=ALU.mult, op1=ALU.add,
                ).then_inc(sem_embB, 1)

                # --- broadcast copies PSUM -> SBUF (bf16) ---
                vector.wait_ge(sem_bcmm, 1)
                vector.tensor_copy(a_fc[:, 0:512], bc_ps[0][:])
                vector.tensor_copy(a_bc[:, 0:512], bc_ps[0][:]).then_inc(sem_bccp, 1)
                vector.wait_ge(sem_bcmm, 2)
                vector.tensor_copy(a_fc[:, 512:1024], bc_ps[1][:])
                vector.tensor_copy(a_bc[:, 512:1024], bc_ps[1][:])
                vector.drain()

                # --- x modulation stream (adds lag one tile: the stt/y of
                # tile i drains tile i-1's zt write, so no per-tile drain)
                def do_add(j):
                    if j >= NY:
                        vector.wait_ge(sem_ot[j % NY], 16 * (j // NY))
                    vector.tensor_tensor(
                        out=wt_sb[:, j % NY, 0:D], in0=zt[j % NZ][:],
                        in1=b_bc[:], op=ALU.add,
                    ).then_inc(sem_add, 1)

                for i in range(NT):
                    if act_owns_z(i):
                        vector.wait_ge(sem_zA, n_zA(i))
                        vector.tensor_tensor(
                            out=zt[i % NZ][:], in0=zt[i % NZ][:], in1=a_bc[:],
                            op=ALU.mult,
                        )
                    else:
                        vector.wait_ge(sem_std, i + 1)
                        vector.scalar_tensor_tensor(
                            out=zt[i % NZ][:], in0=xt[i % NX][:],
                            scalar=inv[i % NS][:], in1=a_fc[:],
                            op0=ALU.mult, op1=ALU.mult,
                        )
                    if i == 0:
                        vector.wait_ge(sem_bcmm, 3)
                        vector.tensor_copy(b_bc[:, 0:512], bc_ps[2][:])
                        vector.wait_ge(sem_bcmm, 4)
                        vector.tensor_copy(b_bc[:, 512:1024], bc_ps[0][:])
                    else:
                        do_add(i - 1)
                do_add(NT - 1)

            # ================= PE =================
            @block.tensor
            def _(tensor):
                tensor.wait_ge(sem_c8, 32)
                tensor.transpose(ct_ps, c8[:], id8[:]).then_inc(sem_tp, 1)
                tensor.wait_ge(sem_sw, 1)
                tensor.matmul(
                    par_ps[:, 0:1], lhsT=ones[:], rhs=sw[:], start=True, stop=True
                ).then_inc(sem_pe2, 1)
                tensor.wait_ge(sem_swcp, 1)
                tensor.wait_ge(sem_r127, 1)
                tensor.matmul(
                    par_ps[:, 1:2], lhsT=ones[0:1, :], rhs=r127[:],
                    start=True, stop=True,
                ).then_inc(sem_pe1, 1)
                tensor.wait_ge(sem_cq, 1)
                tensor.wait_ge(sem_cqs, 1)
                tensor.matmul(
                    par_ps[:, 2:3], lhsT=ones[:], rhs=cqsum[:],
                    start=True, stop=True,
                ).then_inc(sem_pe3, 1)
                tensor.wait_ge(sem_cq, 1)
                for k in range(KC):
                    tensor.wait_ge(sem_wq, k + 1)
                    for n in range(4):
                        mmi = tensor.matmul(
                            emb_ps[:, n, :],
                            lhsT=cqi[:, k : k + 1],
                            rhs=u[k % NU][:, n * 512 : (n + 1) * 512],
                            start=(k == 0),
                            stop=(k == KC - 1),
                        )
                        if k == KC - 1 and n == 1:
                            mmi.then_inc(sem_mmh, 1)
                        if n == 3:
                            mmi.then_inc(sem_mmk, 1)
                tensor.wait_ge(sem_emb, 1)
                tensor.matmul(
                    bc_ps[0][:], lhsT=obf[:], rhs=a_bf[:, 0:512],
                    start=True, stop=True,
                ).then_inc(sem_bcmm, 1)
                tensor.wait_ge(sem_emb, 2)
                tensor.matmul(
                    bc_ps[1][:], lhsT=obf[:], rhs=a_bf[:, 512:1024],
                    start=True, stop=True,
                ).then_inc(sem_bcmm, 1)
                tensor.wait_ge(sem_embB, 1)
                tensor.matmul(
                    bc_ps[2][:], lhsT=obf[:], rhs=s_bf[:, 0:512],
                    start=True, stop=True,
                ).then_inc(sem_bcmm, 1)
                tensor.wait_ge(sem_embB, 2)
                tensor.wait_ge(sem_bccp, 1)
                tensor.matmul(
                    bc_ps[0][:], lhsT=obf[:], rhs=s_bf[:, 512:1024],
                    start=True, stop=True,
                ).then_inc(sem_bcmm, 1)

    return nc


_ID8 = np.eye(8, dtype=np.float32)

_CACHE = {}


def _built(S=S_FULL):
    key = ("nc", S)
    if key not in _CACHE:
        _CACHE[key] = build(S)
    return _CACHE[key]


def kernel(x, c, w_proj, b_proj, rms_weight, _trace=False):
    x = np.ascontiguousarray(np.asarray(x, dtype=np.float32))
    c = np.ascontiguousarray(np.asarray(c, dtype=np.float32))
    w_proj = np.asarray(w_proj, dtype=np.float32)
    b_proj = np.ascontiguousarray(np.asarray(b_proj, dtype=np.float32))
    rms_weight = np.ascontiguousarray(np.asarray(rms_weight, dtype=np.float32))

    nc = _built(x.shape[1])
    wt = np.ascontiguousarray(w_proj.T)

    in_maps = [
        {
            "x": x[i], "c": c[i], "wt": wt, "b": b_proj, "g": rms_weight,
            "id8": _ID8,
        }
        for i in range(B)
    ]
    res = run_bass_kernel_spmd(nc, in_maps, list(range(B)), trace=_trace)
    kernel.last_results = res
    kernel.last_exec_time_ns = res.exec_time_ns
    return np.stack([res.results[i]["out"] for i in range(B)], axis=0)
